# revision 68
# baseline (speedup 1.0000x reference)
"""Fused causal MHA kernel for TRN2, one core = (batch b, head-group g of 8 heads).

Layouts (per core):
  xt   [1024, N]     X[b]^T                 (k on partitions)
  wq/wk/wv [1024, 512] column shard         (k on partitions)
  wo   [512, 1024]   row shard              (dv on partitions)
  maskt [128, 4*512] transposed relative causal mask tiles r=0..3:
        maskt[j, r*512+i] = mask[i, 128*r+j]  (0 / -1e9)
  outt [1024, N]     partial (X attn Wo_g)^T ; host sums the two
        head-group partials per batch and transposes.

On-chip:
  qt/kt per head-pair hp: [128, N]; partitions = (h0 d0-63, h1 d0-63).
  v per seq m-block: [128, 512]; seq on partitions, 8 heads * 64 on free.
  S^T per (hp, c, jb): psum [128, 1024] = h0|h1; j on partitions, i on free.
  PV col-packed: psumO[0:64] = h0 O^T, [64:128] = h1 O^T.
  denom via ones-lhsT matmul into psumD with the same packing, so the
  reciprocal+scale runs lane-aligned on DVE with no partition broadcast.
"""

import numpy as np
import concourse.bass as bass
import concourse.tile as tile
from concourse import bacc, mybir

F32R = mybir.dt.float32r
F32 = mybir.dt.float32
F16 = mybir.dt.float16
AF = mybir.ActivationFunctionType

P = 128
D = 1024
DH = 512  # head-group width: 8 heads * 64
DK = 64
KB = D // P  # 8 k-blocks
MOFF = [0, 128, 384, 768]  # mask prefix offsets per r
NHP = 4  # head-pairs per core


def build(N=2048, interleave=True):
    MB = N // P  # seq 128-blocks
    MC = N // 512  # seq 512-chunks
    nc = bacc.Bacc("TRN2", target_bir_lowering=False, debug=False)

    xt_d = nc.dram_tensor("xt", [D, N], F16, kind="ExternalInput")
    wq_d = nc.dram_tensor("wq", [D, DH], F16, kind="ExternalInput")
    wk_d = nc.dram_tensor("wk", [D, DH], F16, kind="ExternalInput")
    wv_d = nc.dram_tensor("wv", [D, DH], F16, kind="ExternalInput")
    wo_d = nc.dram_tensor("wo", [DH, D], F16, kind="ExternalInput")
    mask_d = nc.dram_tensor("maskt", [P, P], F32, kind="ExternalInput")
    ones_d = nc.dram_tensor("ones16", [P, DK], F16, kind="ExternalInput")
    out_d = nc.dram_tensor("outt", [D, N], F32, kind="ExternalOutput")

    with tile.TileContext(nc) as tc:
        with (
            tc.tile_pool(name="sb", bufs=1) as sb,
            tc.tile_pool(name="ps", bufs=1, space="PSUM") as ps,
        ):
            # ---- persistent tiles ----
            xt = [sb.tile([P, N], F16, tag="xt", bufs=8, name=f"xt{k}") for k in range(KB)]
            wv = [sb.tile([P, DH], F16, tag="sm512", bufs=14, name=f"wv{k}") for k in range(KB)]
            v = [sb.tile([P, 8 * 65], F16, tag="v", bufs=MB, name=f"v{m}") for m in range(MB)]
            ot = [sb.tile([P, N], F16, tag="ot", bufs=NHP, name=f"ot{t}") for t in range(NHP)]
            maskt = sb.tile([P, P], F32, tag="maskt", bufs=1)
            ones = sb.tile([P, DK], F16, tag="ones", bufs=1)

            # k-major for the first chunk: v_proj MM(m, k) only needs wv[k]
            # and xt[k]'s first columns, so the PE starts after ~2 DMAs
            # instead of after the whole 2MB lead block
            for k in range(KB):
                nc.sync.dma_start(wv[k][:], wv_d.ap()[k * P:(k + 1) * P, :])
                nc.sync.dma_start(
                    xt[k][:, 0:512], xt_d.ap()[k * P:(k + 1) * P, 0:512]
                )
            for cc in range(512, N, 512):
                for k in range(KB):
                    nc.sync.dma_start(
                        xt[k][:, cc:cc + 512],
                        xt_d.ap()[k * P:(k + 1) * P, cc:cc + 512],
                    )
            nc.sync.dma_start(maskt[:], mask_d.ap())
            nc.sync.dma_start(ones[:], ones_d.ap())
            # HAM warm-up: ~3.5us of dummy matmuls at t=0 so the PE clock
            # is at 2.4GHz by the time real work (and its DMAs) arrive.
            warmw = sb.tile([P, 512], F16, tag="warmw", bufs=1, name="warmw")
            nc.gpsimd.memset(warmw[:], 0.0)
            psW = ps.tile([P, 512], F32, tag="proj", bufs=2, name="psW")
            psW2 = ps.tile([P, 512], F32, tag="proj", bufs=2, name="psW2")
            for i in range(8):
                nc.tensor.matmul(
                    (psW if i % 2 == 0 else psW2)[:],
                    warmw[:, 0:P], warmw[:], start=True, stop=True,
                )
            warmc = sb.tile([P, 512], F16, tag="warmw", bufs=1, name="warmc")
            nc.vector.tensor_copy(warmc[:], psW[:])
            nc.vector.tensor_copy(warmc[:], psW2[:])
            # warm the ACT exp table set during the DMA lead-in
            warm = sb.tile([P, DK], F16, tag="warm", bufs=1, name="warm")
            nc.scalar.activation(warm[:], ones[:], AF.Exp)
            wo_t = [
                sb.tile([P, D], F16, tag="wo", bufs=NHP, name=f"wo{dv}")
                for dv in range(NHP)
            ]

            # ---- deferred projection work (pumped between attention units) ----
            # entries are (deadline, fn); deadline is a (t, c) chunk key or
            # None. FIFO pump; force_drain emits everything due before a
            # chunk so reads never precede their producer in program order.
            deferred = []
            norm_pending = []  # (chunk seq idx, fn): norm stages, emitted late
            credit = [0.0]
            hold = [0]

            def pump(rate):
                if norm_pending:
                    norm_pending.pop(0)[1]()
                credit[0] += rate
                while credit[0] >= 1.0 and len(deferred) > hold[0]:
                    deferred.pop(0)[1]()
                    credit[0] -= 1.0
                if len(deferred) <= hold[0]:
                    credit[0] = 0.0

            def norm_drain(upto):
                while norm_pending and norm_pending[0][0] <= upto:
                    norm_pending.pop(0)[1]()

            def force_drain(upto):
                while deferred and deferred[0][0] is not None and deferred[0][0] <= upto:
                    deferred.pop(0)[1]()

            def v_proj(m):
                psV = ps.tile([P, 512], F32, tag="proj", bufs=2, name="psV")
                for k in range(KB):
                    nc.tensor.matmul(
                        psV[:],
                        xt[k][:, m * P:(m + 1) * P],
                        wv[k][:],
                        start=(k == 0),
                        stop=(k == KB - 1),
                    )
                v3 = v[m][:].rearrange("p (h x) -> p h x", x=65)
                nc.vector.tensor_copy(
                    v3[:, :, 0:64], psV[:].rearrange("p (h x) -> p h x", x=64)
                )
                nc.vector.tensor_copy(v3[:, :, 64:65], ones[:, 0:8, None])

            def qk_proj_parts(hp, c, w_tiles, dst):
                cell = {}

                def part(k0, k1, fin):
                    if k0 == 0:
                        cell["ps"] = ps.tile(
                            [P, 512], F32, tag="proj", bufs=2, name="psQ"
                        )
                    psQ = cell["ps"]
                    for k in range(k0, k1):
                        nc.tensor.matmul(
                            psQ[:],
                            w_tiles[k][:],
                            xt[k][:, c * 512:(c + 1) * 512],
                            start=(k == 0),
                            stop=(k == KB - 1),
                        )
                    if fin:
                        # scale (1/sqrt(DK)) is folded into the exp ACT's
                        # free affine, so Q and K both finalize as a copy
                        nc.vector.tensor_copy(
                            dst[:, c * 512:(c + 1) * 512], psQ[:]
                        )

                return [
                    lambda: part(0, 4, False),
                    lambda: part(4, KB, True),
                ]

            qt = {}
            kt = {}

            def qk_work(hp):
                qt[hp] = sb.tile([P, N], F16, tag="qt", bufs=4, name=f"qt{hp}")
                kt[hp] = sb.tile([P, N], F16, tag="kt", bufs=4, name=f"kt{hp}")
                wqt = [sb.tile([P, P], F16, tag="wq", bufs=32, name=f"wq{hp}_{k}") for k in range(KB)]
                wkt = [sb.tile([P, P], F16, tag="wk", bufs=32, name=f"wk{hp}_{k}") for k in range(KB)]
                for k in range(KB):
                    nc.sync.dma_start(
                        wqt[k][:], wq_d.ap()[k * P:(k + 1) * P, hp * P:(hp + 1) * P]
                    )
                    nc.sync.dma_start(
                        wkt[k][:], wk_d.ap()[k * P:(k + 1) * P, hp * P:(hp + 1) * P]
                    )
                out = []
                for c in range(MC):
                    for fn in qk_proj_parts(hp, c, wqt, qt[hp]):
                        out.append(((hp, c), fn))
                    for fn in qk_proj_parts(hp, c, wkt, kt[hp]):
                        out.append(((hp, c), fn))
                return out

            def attn_chunk(hp, c, pump_rate=0.5, norm_q=None):
                jb_max = min(MB, 4 * c + 4)
                psOa = [
                    ps.tile([65, 512], F32, tag="psO", bufs=2, name="psO0"),
                    ps.tile([65, 512], F32, tag="psO", bufs=2, name="psO1"),
                ]
                pts = {}

                def stage_s(jb):
                    psS = ps.tile([P, 1024], F32, tag="psS", bufs=2, name="psS")
                    r = jb - 4 * c
                    pre = P * r if r > 0 else 0
                    for h2 in range(2):
                        nc.tensor.matmul(
                            psS[:, h2 * 512 + pre:(h2 + 1) * 512],
                            kt[hp][h2 * DK:(h2 + 1) * DK, jb * P:(jb + 1) * P],
                            qt[hp][h2 * DK:(h2 + 1) * DK, c * 512 + pre:(c + 1) * 512],
                            start=True,
                            stop=True,
                            tile_position=(h2 * DK, 0),
                        )
                    if r >= 0:
                        # only the 128-wide triangle needs the additive mask;
                        # columns below the prefix are fully masked and are
                        # zeroed in pt after the exp instead
                        for h2 in range(2):
                            nc.vector.tensor_add(
                                psS[:, h2 * 512 + pre:h2 * 512 + pre + P],
                                psS[:, h2 * 512 + pre:h2 * 512 + pre + P],
                                maskt[:],
                            )
                    pt = sb.tile([P, 1024], F16, tag="pt", bufs=4, name="pt")
                    # 1/sqrt(DK) rides the ACT's free affine (scale); the
                    # masked prefix is never read by PV so it's left stale.
                    if pre:
                        # one strided ACT over both heads' valid slices
                        psS3 = psS[:].rearrange("p (h x) -> p h x", h=2)
                        pt3 = pt[:].rearrange("p (h x) -> p h x", h=2)
                        nc.scalar.activation(
                            pt3[:, :, pre:512], psS3[:, :, pre:512], AF.Exp,
                            scale=0.125,
                        )
                    else:
                        nc.scalar.activation(pt[:], psS[:], AF.Exp, scale=0.125)
                    pts[jb] = pt

                def stage_pv(jb):
                    pt = pts.pop(jb)
                    first, last = (jb == 0), (jb == jb_max - 1)
                    r = jb - 4 * c
                    pre = P * r if (r > 0 and not first) else 0
                    for h2 in range(2):
                        h = 2 * hp + h2
                        nc.tensor.matmul(
                            psOa[h2][0:65, pre:512],
                            v[jb][:, h * 65:(h + 1) * 65],
                            pt[:, h2 * 512 + pre:(h2 + 1) * 512],
                            start=first,
                            stop=last,
                            skip_group_check=True,
                        )
                    pump(pump_rate)

                # fire the pump once up front: the S->exp windup otherwise
                # idles the PE before the first stage_pv-driven pop
                pump(pump_rate)
                for jb in range(jb_max):
                    stage_s(jb)
                    if jb >= 2:
                        stage_pv(jb - 2)
                stage_pv(jb_max - 2)
                stage_pv(jb_max - 1)

                cpO = [
                    sb.tile([65, 512], F32, tag="sm512", bufs=14, name=f"cpO{h2}")
                    for h2 in range(2)
                ]
                nc.vector.tensor_copy(cpO[0][0:65, :], psOa[0][0:65, :])
                nc.vector.tensor_copy(cpO[1][0:65, :], psOa[1][0:65, :])
                rbc = [
                    sb.tile([64, 512], F32, tag="sm512", bufs=14, name=f"rbc{h2}")
                    for h2 in range(2)
                ]
                tmp1 = sb.tile([64, 512], F16, tag="sm512", bufs=14, name="tmp1")

                nr = sb.tile([1, 1024], F32, tag="nr", bufs=4, name="nr")
                nr2 = sb.tile([1, 1024], F32, tag="nr", bufs=4, name="nr2")

                def norm_piece(stage):
                    if stage == 0:
                        # move denominator rows (lane 64) to lane 0
                        nc.sync.dma_start(nr[0:1, 0:512], cpO[0][64:65, :])
                        nc.sync.dma_start(nr[0:1, 512:1024], cpO[1][64:65, :])
                    elif stage == 1:
                        nc.vector.reciprocal_approx_fast(nr2[0:1, :], nr[0:1, :])
                    elif stage == 2:
                        nc.gpsimd.partition_broadcast(
                            rbc[0][0:64, :], nr2[0:1, 0:512]
                        )
                        nc.gpsimd.partition_broadcast(
                            rbc[1][0:64, :], nr2[0:1, 512:1024]
                        )
                    elif stage == 3:
                        nc.vector.tensor_tensor(
                            ot[hp][0:64, c * 512:(c + 1) * 512],
                            cpO[0][0:64, :],
                            rbc[0][0:64, :],
                            mybir.AluOpType.mult,
                        )
                    elif stage == 4:
                        nc.vector.tensor_tensor(
                            tmp1[0:64, :],
                            cpO[1][0:64, :],
                            rbc[1][0:64, :],
                            mybir.AluOpType.mult,
                        )
                        nc.sync.dma_start(
                            ot[hp][64:128, c * 512:(c + 1) * 512], tmp1[0:64, :]
                        )

                if not interleave:
                    for st in range(5):
                        norm_piece(st)
                else:
                    for st in range(5):
                        norm_pending.append((norm_q, lambda st=st: norm_piece(st)))

            # ---- schedule ----

            def outproj_parts(do, c, nkey):
                cell = {}

                def part(v0, v1, fin):
                    if v0 == 0:
                        # ot[*][c] writers (norm stages) must be emitted
                        # before any read of them enters the program order
                        norm_drain(nkey)
                        cell["ps"] = ps.tile(
                            [P, 512], F32, tag="proj", bufs=2, name="psF"
                        )
                    psF = cell["ps"]
                    for dv in range(v0, v1):
                        nc.tensor.matmul(
                            psF[:],
                            wo_t[dv][:, do * P:(do + 1) * P],
                            ot[dv][:, c * 512:(c + 1) * 512],
                            start=(dv == 0),
                            stop=(dv == NHP - 1),
                        )
                    if fin:
                        o_sb = sb.tile([P, 512], F32, tag="sm512", bufs=14, name="o_sb")
                        nc.vector.tensor_copy(o_sb[:], psF[:])
                        nc.sync.dma_start(
                            out_d.ap()[do * P:(do + 1) * P, c * 512:(c + 1) * 512],
                            o_sb[:],
                        )

                return [lambda: part(0, 2, False), lambda: part(2, NHP, True)]

            def units_in(chunks):
                return sum(min(MB, 4 * cc + 4) for cc in chunks)

            for dv in range(NHP):
                nc.sync.dma_start(wo_t[dv][:], wo_d.ap()[dv * P:(dv + 1) * P, :])
            qk0 = qk_work(0)
            for m in range(MB):
                v_proj(m)
            for _, fn in qk0:
                fn()

            # hp0 runs chunk-sequential (its K/Q arrive first); hp1-3 go
            # chunk-outer so each (3, c) releases outproj(c) parts as PE
            # filler early in the remaining ACT-bound attention stream,
            # instead of piling every outproj into hp3's span.
            # hp0/hp1 chunk-sequential; hp2/hp3 interleave pairwise so each
            # (3, c) releases outproj(c) parts as PE filler for the
            # remaining ACT-bound attention
            seq = [(0, c) for c in range(MC)] + [(1, c) for c in range(MC)]
            seq += [(t, c) for c in range(MC) for t in (2, 3)]
            pos = {tc: i for i, tc in enumerate(seq)}

            if not interleave:
                for t in range(NHP):
                    for _, fn in (qk_work(t + 1) if t + 1 < NHP else []):
                        fn()
                    for c in range(MC):
                        attn_chunk(t, c)
                for c in range(MC):
                    for do in range(D // P):
                        for th in outproj_parts(do, c, 0):
                            th()
            else:
                for idx, (t, c) in enumerate(seq):
                    if (t, c) == (0, 0):
                        deferred.extend(
                            (pos[dl], fn) for dl, fn in qk_work(1)
                        )
                    elif (t, c) == (1, 0):
                        merged = sorted(
                            [(pos[dl], fn) for dl, fn in qk_work(2)]
                            + [(pos[dl], fn) for dl, fn in qk_work(3)],
                            key=lambda e: e[0],
                        )
                        deferred.extend(merged)
                    force_drain(idx)
                    rem = units_in(cc for _, cc in seq[idx:])
                    rate = min(
                        3.0,
                        len(deferred) / max(rem - 8, 1) + 0.15,
                    )
                    attn_chunk(t, c, pump_rate=rate, norm_q=idx)
                    if t == 3:
                        if (t, c) == seq[-1]:
                            # final chunk: emit its norm chain right away so
                            # the tail latency starts as early as possible
                            norm_drain(idx)
                        for do in range(D // P):
                            for fn in outproj_parts(do, c, idx):
                                deferred.append((None, fn))

            # ---- drain remaining deferred work ----
            norm_drain(len(seq))
            while deferred:
                deferred.pop(0)[1]()

    nc.compile()
    return nc


def make_core_inputs(X, mask, Wq, Wk, Wv, Wo):
    """Full inputs -> list of 8 per-core input maps (batch-major, head-group minor)."""
    B = X.shape[0]
    maskt = np.ascontiguousarray(mask[0:P, 0:P].T.astype(np.float32))
    in_maps = []
    for b in range(B):
        xt = np.ascontiguousarray(X[b].T.astype(np.float16))
        for g in range(2):
            sl = slice(g * DH, (g + 1) * DH)
            in_maps.append(
                {
                    "xt": xt,
                    "wq": np.ascontiguousarray(Wq[:, sl].astype(np.float16)),
                    "wk": np.ascontiguousarray(Wk[:, sl].astype(np.float16)),
                    "wv": np.ascontiguousarray(Wv[:, sl].astype(np.float16)),
                    "wo": np.ascontiguousarray(Wo[sl, :].astype(np.float16)),
                    "maskt": maskt,
                    "ones16": np.ones((P, DK), np.float16),
                }
            )
    return in_maps


def gather_output(results, B=4):
    N = results[0]["outt"].shape[1]
    out = np.empty((B, N, D), np.float32)
    for b in range(B):
        s = results[2 * b]["outt"] + results[2 * b + 1]["outt"]
        out[b] = s.T
    return out


# ---------------------------------------------------------------------------
# Self-contained harness entry: full inputs in, full output out.
# Shards across 8 NeuronCores: core = batch b (4) x head-group g (2 x 8 heads).
# Each core runs a fused flash-style causal MHA for its 8 heads; the host
# sums the two head-group partial outputs per batch (row-parallel W_O).
# ---------------------------------------------------------------------------
from concourse.bass_utils import run_bass_kernel_spmd

_NC_CACHE = {}


def _get_nc():
    if "nc" not in _NC_CACHE:
        _NC_CACHE["nc"] = build(N=2048, interleave=True)
    return _NC_CACHE["nc"]


def kernel(X, mask, Wq, Wk, Wv, Wo):
    X = np.asarray(X, dtype=np.float32)
    mask = np.asarray(mask, dtype=np.float32)
    Wq = np.asarray(Wq, dtype=np.float32)
    Wk = np.asarray(Wk, dtype=np.float32)
    Wv = np.asarray(Wv, dtype=np.float32)
    Wo = np.asarray(Wo, dtype=np.float32)
    in_maps = make_core_inputs(X, mask, Wq, Wk, Wv, Wo)
    nc = _get_nc()
    res = run_bass_kernel_spmd(nc, in_maps, list(range(8)))
    return gather_output(res.results, B=X.shape[0])



# revision 69
# speedup vs baseline: 1.0079x; 1.0079x over previous
"""Fused causal MHA kernel for TRN2, one core = (batch b, head-group g of 8 heads).

Layouts (per core):
  xt   [1024, N]     X[b]^T                 (k on partitions)
  wq/wk/wv [1024, 512] column shard         (k on partitions)
  wo   [512, 1024]   row shard              (dv on partitions)
  maskt [128, 4*512] transposed relative causal mask tiles r=0..3:
        maskt[j, r*512+i] = mask[i, 128*r+j]  (0 / -1e9)
  outt [1024, N]     partial (X attn Wo_g)^T ; host sums the two
        head-group partials per batch and transposes.

On-chip:
  qt/kt per head-pair hp: [128, N]; partitions = (h0 d0-63, h1 d0-63).
  v per seq m-block: [128, 512]; seq on partitions, 8 heads * 64 on free.
  S^T per (hp, c, jb): psum [128, 1024] = h0|h1; j on partitions, i on free.
  PV col-packed: psumO[0:64] = h0 O^T, [64:128] = h1 O^T.
  denom via ones-lhsT matmul into psumD with the same packing, so the
  reciprocal+scale runs lane-aligned on DVE with no partition broadcast.
"""

import numpy as np
import concourse.bass as bass
import concourse.tile as tile
from concourse import bacc, mybir

F32R = mybir.dt.float32r
F32 = mybir.dt.float32
F16 = mybir.dt.float16
AF = mybir.ActivationFunctionType

P = 128
D = 1024
DH = 512  # head-group width: 8 heads * 64
DK = 64
KB = D // P  # 8 k-blocks
MOFF = [0, 128, 384, 768]  # mask prefix offsets per r
NHP = 4  # head-pairs per core


def build(N=2048, interleave=True):
    MB = N // P  # seq 128-blocks
    MC = N // 512  # seq 512-chunks
    nc = bacc.Bacc("TRN2", target_bir_lowering=False, debug=False)

    xt_d = nc.dram_tensor("xt", [D, N], F16, kind="ExternalInput")
    wq_d = nc.dram_tensor("wq", [D, DH], F16, kind="ExternalInput")
    wk_d = nc.dram_tensor("wk", [D, DH], F16, kind="ExternalInput")
    wv_d = nc.dram_tensor("wv", [D, DH], F16, kind="ExternalInput")
    wo_d = nc.dram_tensor("wo", [DH, D], F16, kind="ExternalInput")
    mask_d = nc.dram_tensor("maskt", [P, P], F32, kind="ExternalInput")
    ones_d = nc.dram_tensor("ones16", [P, DK], F16, kind="ExternalInput")
    out_d = nc.dram_tensor("outt", [D, N], F32, kind="ExternalOutput")

    with tile.TileContext(nc) as tc:
        with (
            tc.tile_pool(name="sb", bufs=1) as sb,
            tc.tile_pool(name="ps", bufs=1, space="PSUM") as ps,
        ):
            # ---- persistent tiles ----
            xt = [sb.tile([P, N], F16, tag="xt", bufs=8, name=f"xt{k}") for k in range(KB)]
            wv = [sb.tile([P, DH], F16, tag="sm512", bufs=14, name=f"wv{k}") for k in range(KB)]
            v = [sb.tile([P, 8 * 65], F16, tag="v", bufs=MB, name=f"v{m}") for m in range(MB)]
            ot = [sb.tile([P, N], F16, tag="ot", bufs=NHP, name=f"ot{t}") for t in range(NHP)]
            maskt = sb.tile([P, P], F32, tag="maskt", bufs=1)
            ones = sb.tile([P, DK], F16, tag="ones", bufs=1)

            # k-major for the first chunk: v_proj MM(m, k) only needs wv[k]
            # and xt[k]'s first columns, so the PE starts after ~2 DMAs
            # instead of after the whole 2MB lead block
            for k in range(KB):
                nc.sync.dma_start(wv[k][:], wv_d.ap()[k * P:(k + 1) * P, :])
                nc.sync.dma_start(
                    xt[k][:, 0:512], xt_d.ap()[k * P:(k + 1) * P, 0:512]
                )
            for cc in range(512, N, 512):
                for k in range(KB):
                    nc.sync.dma_start(
                        xt[k][:, cc:cc + 512],
                        xt_d.ap()[k * P:(k + 1) * P, cc:cc + 512],
                    )
            nc.sync.dma_start(maskt[:], mask_d.ap())
            nc.sync.dma_start(ones[:], ones_d.ap())
            # HAM warm-up: ~3.5us of dummy matmuls at t=0 so the PE clock
            # is at 2.4GHz by the time real work (and its DMAs) arrive.
            warmw = sb.tile([P, 512], F16, tag="warmw", bufs=1, name="warmw")
            nc.gpsimd.memset(warmw[:], 0.0)
            psW = ps.tile([P, 512], F32, tag="proj", bufs=2, name="psW")
            psW2 = ps.tile([P, 512], F32, tag="proj", bufs=2, name="psW2")
            for i in range(8):
                nc.tensor.matmul(
                    (psW if i % 2 == 0 else psW2)[:],
                    warmw[:, 0:P], warmw[:], start=True, stop=True,
                )
            warmc = sb.tile([P, 512], F16, tag="warmw", bufs=1, name="warmc")
            nc.vector.tensor_copy(warmc[:], psW[:])
            nc.vector.tensor_copy(warmc[:], psW2[:])
            # warm the ACT exp table set during the DMA lead-in
            warm = sb.tile([P, DK], F16, tag="warm", bufs=1, name="warm")
            nc.scalar.activation(warm[:], ones[:], AF.Exp)
            wo_t = [
                sb.tile([P, D], F16, tag="wo", bufs=NHP, name=f"wo{dv}")
                for dv in range(NHP)
            ]

            # ---- deferred projection work (pumped between attention units) ----
            # entries are (deadline, fn); deadline is a (t, c) chunk key or
            # None. FIFO pump; force_drain emits everything due before a
            # chunk so reads never precede their producer in program order.
            deferred = []
            norm_pending = []  # (chunk seq idx, fn): norm stages, emitted late
            credit = [0.0]
            hold = [0]

            def pump(rate):
                if norm_pending:
                    norm_pending.pop(0)[1]()
                credit[0] += rate
                while credit[0] >= 1.0 and len(deferred) > hold[0]:
                    deferred.pop(0)[1]()
                    credit[0] -= 1.0
                if len(deferred) <= hold[0]:
                    credit[0] = 0.0

            def norm_drain(upto):
                while norm_pending and norm_pending[0][0] <= upto:
                    norm_pending.pop(0)[1]()

            def force_drain(upto):
                while deferred and deferred[0][0] is not None and deferred[0][0] <= upto:
                    deferred.pop(0)[1]()

            def v_proj(m):
                psV = ps.tile([P, 512], F32, tag="proj", bufs=2, name="psV")
                for k in range(KB):
                    nc.tensor.matmul(
                        psV[:],
                        xt[k][:, m * P:(m + 1) * P],
                        wv[k][:],
                        start=(k == 0),
                        stop=(k == KB - 1),
                    )
                v3 = v[m][:].rearrange("p (h x) -> p h x", x=65)
                nc.vector.tensor_copy(
                    v3[:, :, 0:64], psV[:].rearrange("p (h x) -> p h x", x=64)
                )
                nc.vector.tensor_copy(v3[:, :, 64:65], ones[:, 0:8, None])

            def qk_proj_parts(hp, c, w_tiles, dst):
                cell = {}

                def part(k0, k1, fin):
                    if k0 == 0:
                        cell["ps"] = ps.tile(
                            [P, 512], F32, tag="proj", bufs=2, name="psQ"
                        )
                    psQ = cell["ps"]
                    for k in range(k0, k1):
                        nc.tensor.matmul(
                            psQ[:],
                            w_tiles[k][:],
                            xt[k][:, c * 512:(c + 1) * 512],
                            start=(k == 0),
                            stop=(k == KB - 1),
                        )
                    if fin:
                        # scale (1/sqrt(DK)) is folded into the exp ACT's
                        # free affine, so Q and K both finalize as a copy
                        nc.vector.tensor_copy(
                            dst[:, c * 512:(c + 1) * 512], psQ[:]
                        )

                return [
                    lambda: part(0, 4, False),
                    lambda: part(4, KB, True),
                ]

            qt = {}
            kt = {}

            def qk_work(hp):
                qt[hp] = sb.tile([P, N], F16, tag="qt", bufs=4, name=f"qt{hp}")
                kt[hp] = sb.tile([P, N], F16, tag="kt", bufs=4, name=f"kt{hp}")
                wqt = [sb.tile([P, P], F16, tag="wq", bufs=32, name=f"wq{hp}_{k}") for k in range(KB)]
                wkt = [sb.tile([P, P], F16, tag="wk", bufs=32, name=f"wk{hp}_{k}") for k in range(KB)]
                for k in range(KB):
                    nc.sync.dma_start(
                        wqt[k][:], wq_d.ap()[k * P:(k + 1) * P, hp * P:(hp + 1) * P]
                    )
                    nc.sync.dma_start(
                        wkt[k][:], wk_d.ap()[k * P:(k + 1) * P, hp * P:(hp + 1) * P]
                    )
                out = []
                for c in range(MC):
                    for fn in qk_proj_parts(hp, c, wqt, qt[hp]):
                        out.append(((hp, c), fn))
                    for fn in qk_proj_parts(hp, c, wkt, kt[hp]):
                        out.append(((hp, c), fn))
                return out

            def attn_chunk(hp, c, pump_rate=0.5, norm_q=None):
                jb_max = min(MB, 4 * c + 4)
                psOa = [
                    ps.tile([65, 512], F32, tag="psO", bufs=2, name="psO0"),
                    ps.tile([65, 512], F32, tag="psO", bufs=2, name="psO1"),
                ]
                pts = {}

                def stage_s(jb):
                    psS = ps.tile([P, 1024], F32, tag="psS", bufs=2, name="psS")
                    r = jb - 4 * c
                    pre = P * r if r > 0 else 0
                    for h2 in range(2):
                        nc.tensor.matmul(
                            psS[:, h2 * 512 + pre:(h2 + 1) * 512],
                            kt[hp][h2 * DK:(h2 + 1) * DK, jb * P:(jb + 1) * P],
                            qt[hp][h2 * DK:(h2 + 1) * DK, c * 512 + pre:(c + 1) * 512],
                            start=True,
                            stop=True,
                            tile_position=(h2 * DK, 0),
                        )
                    if r >= 0:
                        # only the 128-wide triangle needs the additive mask;
                        # columns below the prefix are fully masked and are
                        # zeroed in pt after the exp instead
                        for h2 in range(2):
                            nc.vector.tensor_add(
                                psS[:, h2 * 512 + pre:h2 * 512 + pre + P],
                                psS[:, h2 * 512 + pre:h2 * 512 + pre + P],
                                maskt[:],
                            )
                    pt = sb.tile([P, 1024], F16, tag="pt", bufs=4, name="pt")
                    # 1/sqrt(DK) rides the ACT's free affine (scale); the
                    # masked prefix is never read by PV so it's left stale.
                    if pre:
                        # one strided ACT over both heads' valid slices
                        psS3 = psS[:].rearrange("p (h x) -> p h x", h=2)
                        pt3 = pt[:].rearrange("p (h x) -> p h x", h=2)
                        nc.scalar.activation(
                            pt3[:, :, pre:512], psS3[:, :, pre:512], AF.Exp,
                            scale=0.125,
                        )
                    else:
                        nc.scalar.activation(pt[:], psS[:], AF.Exp, scale=0.125)
                    pts[jb] = pt

                def stage_pv(jb):
                    pt = pts.pop(jb)
                    first, last = (jb == 0), (jb == jb_max - 1)
                    r = jb - 4 * c
                    pre = P * r if (r > 0 and not first) else 0
                    for h2 in range(2):
                        h = 2 * hp + h2
                        nc.tensor.matmul(
                            psOa[h2][0:65, pre:512],
                            v[jb][:, h * 65:(h + 1) * 65],
                            pt[:, h2 * 512 + pre:(h2 + 1) * 512],
                            start=first,
                            stop=last,
                            skip_group_check=True,
                        )
                    pump(pump_rate)

                for jb in range(jb_max):
                    stage_s(jb)
                    if jb >= 2:
                        stage_pv(jb - 2)
                stage_pv(jb_max - 2)
                stage_pv(jb_max - 1)

                cpO = [
                    sb.tile([65, 512], F32, tag="sm512", bufs=14, name=f"cpO{h2}")
                    for h2 in range(2)
                ]
                nc.vector.tensor_copy(cpO[0][0:65, :], psOa[0][0:65, :])
                nc.vector.tensor_copy(cpO[1][0:65, :], psOa[1][0:65, :])
                rbc = [
                    sb.tile([64, 512], F32, tag="sm512", bufs=14, name=f"rbc{h2}")
                    for h2 in range(2)
                ]
                tmp1 = sb.tile([64, 512], F16, tag="sm512", bufs=14, name="tmp1")

                nr = sb.tile([1, 1024], F32, tag="nr", bufs=4, name="nr")
                nr2 = sb.tile([1, 1024], F32, tag="nr", bufs=4, name="nr2")

                def norm_piece(stage):
                    if stage == 0:
                        # move denominator rows (lane 64) to lane 0
                        nc.sync.dma_start(nr[0:1, 0:512], cpO[0][64:65, :])
                        nc.sync.dma_start(nr[0:1, 512:1024], cpO[1][64:65, :])
                    elif stage == 1:
                        nc.vector.reciprocal_approx_fast(nr2[0:1, :], nr[0:1, :])
                    elif stage == 2:
                        nc.gpsimd.partition_broadcast(
                            rbc[0][0:64, :], nr2[0:1, 0:512]
                        )
                        nc.gpsimd.partition_broadcast(
                            rbc[1][0:64, :], nr2[0:1, 512:1024]
                        )
                    elif stage == 3:
                        nc.vector.tensor_tensor(
                            ot[hp][0:64, c * 512:(c + 1) * 512],
                            cpO[0][0:64, :],
                            rbc[0][0:64, :],
                            mybir.AluOpType.mult,
                        )
                    elif stage == 4:
                        nc.vector.tensor_tensor(
                            tmp1[0:64, :],
                            cpO[1][0:64, :],
                            rbc[1][0:64, :],
                            mybir.AluOpType.mult,
                        )
                        nc.sync.dma_start(
                            ot[hp][64:128, c * 512:(c + 1) * 512], tmp1[0:64, :]
                        )

                if not interleave:
                    for st in range(5):
                        norm_piece(st)
                else:
                    for st in range(5):
                        norm_pending.append((norm_q, lambda st=st: norm_piece(st)))

            # ---- schedule ----

            def outproj_parts(do, c, nkey):
                cell = {}

                def part(v0, v1, fin):
                    if v0 == 0:
                        # ot[*][c] writers (norm stages) must be emitted
                        # before any read of them enters the program order
                        norm_drain(nkey)
                        cell["ps"] = ps.tile(
                            [P, 512], F32, tag="proj", bufs=2, name="psF"
                        )
                    psF = cell["ps"]
                    for dv in range(v0, v1):
                        nc.tensor.matmul(
                            psF[:],
                            wo_t[dv][:, do * P:(do + 1) * P],
                            ot[dv][:, c * 512:(c + 1) * 512],
                            start=(dv == 0),
                            stop=(dv == NHP - 1),
                        )
                    if fin:
                        o_sb = sb.tile([P, 512], F32, tag="sm512", bufs=14, name="o_sb")
                        nc.vector.tensor_copy(o_sb[:], psF[:])
                        nc.sync.dma_start(
                            out_d.ap()[do * P:(do + 1) * P, c * 512:(c + 1) * 512],
                            o_sb[:],
                        )

                return [lambda: part(0, 2, False), lambda: part(2, NHP, True)]

            def units_in(chunks):
                return sum(min(MB, 4 * cc + 4) for cc in chunks)

            for dv in range(NHP):
                nc.sync.dma_start(wo_t[dv][:], wo_d.ap()[dv * P:(dv + 1) * P, :])
            qk0 = qk_work(0)
            for m in range(MB):
                v_proj(m)
            for _, fn in qk0:
                fn()

            # hp0 runs chunk-sequential (its K/Q arrive first); hp1-3 go
            # chunk-outer so each (3, c) releases outproj(c) parts as PE
            # filler early in the remaining ACT-bound attention stream,
            # instead of piling every outproj into hp3's span.
            # hp0/hp1 chunk-sequential; hp2/hp3 interleave pairwise so each
            # (3, c) releases outproj(c) parts as PE filler for the
            # remaining ACT-bound attention
            seq = [(0, c) for c in range(MC)] + [(1, c) for c in range(MC)]
            seq += [(t, c) for c in range(MC) for t in (2, 3)]
            pos = {tc: i for i, tc in enumerate(seq)}

            if not interleave:
                for t in range(NHP):
                    for _, fn in (qk_work(t + 1) if t + 1 < NHP else []):
                        fn()
                    for c in range(MC):
                        attn_chunk(t, c)
                for c in range(MC):
                    for do in range(D // P):
                        for th in outproj_parts(do, c, 0):
                            th()
            else:
                for idx, (t, c) in enumerate(seq):
                    if (t, c) == (0, 0):
                        deferred.extend(
                            (pos[dl], fn) for dl, fn in qk_work(1)
                        )
                    elif (t, c) == (1, 0):
                        merged = sorted(
                            [(pos[dl], fn) for dl, fn in qk_work(2)]
                            + [(pos[dl], fn) for dl, fn in qk_work(3)],
                            key=lambda e: e[0],
                        )
                        deferred.extend(merged)
                    force_drain(idx)
                    rem = units_in(cc for _, cc in seq[idx:])
                    rate = min(
                        3.0,
                        len(deferred) / max(rem - 8, 1) + 0.15,
                    )
                    attn_chunk(t, c, pump_rate=rate, norm_q=idx)
                    if t == 3:
                        if (t, c) == seq[-1]:
                            # final chunk: emit its norm chain right away so
                            # the tail latency starts as early as possible
                            norm_drain(idx)
                        for do in range(D // P):
                            for fn in outproj_parts(do, c, idx):
                                deferred.append((None, fn))

            # ---- drain remaining deferred work ----
            norm_drain(len(seq))
            while deferred:
                deferred.pop(0)[1]()

    nc.compile()
    return nc


def make_core_inputs(X, mask, Wq, Wk, Wv, Wo):
    """Full inputs -> list of 8 per-core input maps (batch-major, head-group minor)."""
    B = X.shape[0]
    maskt = np.ascontiguousarray(mask[0:P, 0:P].T.astype(np.float32))
    in_maps = []
    for b in range(B):
        xt = np.ascontiguousarray(X[b].T.astype(np.float16))
        for g in range(2):
            sl = slice(g * DH, (g + 1) * DH)
            in_maps.append(
                {
                    "xt": xt,
                    "wq": np.ascontiguousarray(Wq[:, sl].astype(np.float16)),
                    "wk": np.ascontiguousarray(Wk[:, sl].astype(np.float16)),
                    "wv": np.ascontiguousarray(Wv[:, sl].astype(np.float16)),
                    "wo": np.ascontiguousarray(Wo[sl, :].astype(np.float16)),
                    "maskt": maskt,
                    "ones16": np.ones((P, DK), np.float16),
                }
            )
    return in_maps


def gather_output(results, B=4):
    N = results[0]["outt"].shape[1]
    out = np.empty((B, N, D), np.float32)
    for b in range(B):
        s = results[2 * b]["outt"] + results[2 * b + 1]["outt"]
        out[b] = s.T
    return out


# ---------------------------------------------------------------------------
# Self-contained harness entry: full inputs in, full output out.
# Shards across 8 NeuronCores: core = batch b (4) x head-group g (2 x 8 heads).
# Each core runs a fused flash-style causal MHA for its 8 heads; the host
# sums the two head-group partial outputs per batch (row-parallel W_O).
# ---------------------------------------------------------------------------
from concourse.bass_utils import run_bass_kernel_spmd

_NC_CACHE = {}


def _get_nc():
    if "nc" not in _NC_CACHE:
        _NC_CACHE["nc"] = build(N=2048, interleave=True)
    return _NC_CACHE["nc"]


def kernel(X, mask, Wq, Wk, Wv, Wo):
    X = np.asarray(X, dtype=np.float32)
    mask = np.asarray(mask, dtype=np.float32)
    Wq = np.asarray(Wq, dtype=np.float32)
    Wk = np.asarray(Wk, dtype=np.float32)
    Wv = np.asarray(Wv, dtype=np.float32)
    Wo = np.asarray(Wo, dtype=np.float32)
    in_maps = make_core_inputs(X, mask, Wq, Wk, Wv, Wo)
    nc = _get_nc()
    res = run_bass_kernel_spmd(nc, in_maps, list(range(8)))
    return gather_output(res.results, B=X.shape[0])

